# revision 2
# baseline (speedup 1.0000x reference)
"""AttentionBlock (GroupNorm + single-head self-attention + residual) as a
Bass/Tile kernel for one Trainium2 chip (8 NeuronCores), SPMD data-parallel.

fp8/bf16 redesign of the f32r baseline:
- QKV projections run in bf16 (x and weights bf16; fp8 here would push the
  worst-case rel err near the 2e-2 gate on wide-score datasets). Their
  outputs are cast to fp8e4 at PSUM evacuation.
- Attention matmuls (scores, PV, out-proj) run in float8e4 (e4m3) with
  MatmulPerfMode.DoubleRow: the 256-deep contraction is packed as two
  128-row matrices per call at 0.5 PE cycles/row.
- Softmax over the partition axis (S^T layout): exp on ACT in [128,1024]
  batches straight out of paired PSUM banks; exp arg is SCALE*s - 3.5 so
  fp8 pt can't overflow (ACT's fp8 cast yields inf, not saturation, above
  240; shift-invariance keeps softmax exact). The denominator is an
  all-ones DoubleRow matmul accumulated on PE alongside PV — it reduces
  over j AND broadcasts the sum to all 128 partitions (replaces the
  baseline's DVE accumulation tree + ones_sq broadcast matmul).
- GroupNorm stats come from every other column of the bf16 x copy
  (sampling error ~0.3% of sigma, far below tolerance) so the stats chain
  clears ~4us after the x DMA. rsqrt = ACT Sqrt + DVE reciprocal;
  identity/copy live in every ACT table so the only table switch is
  sqrt -> exp, once.
- V's bias is never applied on-chip: sum_j attn = 1, so it folds into the
  output-projection bias (b_fp = bp + wp.T @ (bv + wv.T @ gn_bias)).
- K (g-major) and Q stream group-wise; attention chunk 0 starts once the
  first K/Q groups land, and the remaining K/Q/V work is interleaved into
  the chunk-0/1 pair slots (PE has ~2x slack under the ACT exp stream).

Sharding: 4 images x 2 query-halves -> 8 cores. x is pre-rolled per half
on the host (keys are permutation-invariant); residual/output use the
original column range h*L..(h+1)*L.
"""

import numpy as np

import bass_rust
import concourse.bass as bass
import concourse.mybir as mybir
import concourse.tile as tile
from concourse.bass import ts
from concourse.bass_utils import run_bass_kernel_spmd

# ---------------------------------------------------------------------------
# walrus single-sync-wait workaround (same as baseline)

_counter = [0]


def _mk_nop(engine, wait):
    _counter[0] += 1
    nop = mybir.InstNoOp(name=f"WSPLIT-{_counter[0]}", ins=[], outs=[])
    nop.engine = engine
    nop.sync_info = bass_rust.SyncInfo(on_wait=[wait], on_update=[])
    return nop


def split_waits(nc, verbose=False):
    f = nc.m.functions[0]
    new_blocks = []
    n_split = 0
    for blk in f.blocks:
        insts = blk.instructions
        out = []
        for inst in insts:
            si = inst.sync_info
            if si is not None and si.on_wait and len(si.on_wait) > 1:
                waits = list(si.on_wait)
                for w in waits[1:]:
                    out.append(_mk_nop(inst.engine, w))
                si.on_wait = waits[:1]
                n_split += 1
            out.append(inst)
        new_blocks.append(bass_rust.BasicBlock(name=blk.name, instructions=out))
    f.blocks = new_blocks
    if verbose:
        print(f"split_waits: split {n_split} instructions")
    return n_split


# ---------------------------------------------------------------------------

DT = mybir.dt.float32
DB = mybir.dt.bfloat16
D8 = mybir.dt.float8e4
AF = mybir.ActivationFunctionType
OP = mybir.AluOpType
DRM = mybir.MatmulPerfMode.DoubleRow

C = 256
N = 4096
L = 2048
IC = 512          # i-chunk size
NCH = L // IC     # 4 chunks
NJT = N // 128    # 32 j-tiles
NJP = NJT // 2    # 16 j-tile pairs
CT = C // 128     # 2 channel tiles
GROUPS = 8
EPS = 1e-5
SCALE = C ** -0.5
SHIFT = -3.5


def build(split=True, repeat=1, prec=None, debug=False):
    nc = bass.Bass()
    dbg_d = {}
    if debug:
        dbg_d["d_scale"] = nc.declare_dram_parameter("d_scale", [128, 2 * CT], DT, isOutput=True)
        dbg_d["d_k8"] = nc.declare_dram_parameter("d_k8", [128, CT * N], DT, isOutput=True)
        dbg_d["d_q8"] = nc.declare_dram_parameter("d_q8", [128, CT * L], DT, isOutput=True)
        dbg_d["d_vt8"] = nc.declare_dram_parameter("d_vt8", [128, 2 * 512], DT, isOutput=True)
        dbg_d["d_pt"] = nc.declare_dram_parameter("d_pt", [128, 1024], DT, isOutput=True)
        dbg_d["d_den"] = nc.declare_dram_parameter("d_den", [128, IC], DT, isOutput=True)
        dbg_d["d_ao"] = nc.declare_dram_parameter("d_ao", [128, 2 * IC], DT, isOutput=True)

    # xb: full image, bf16, ct-major free dim: [p, t*N + j] = x[t*128+p, j]
    xb_d = nc.declare_dram_parameter("xb", [128, CT * N], DB, isOutput=False)
    # x8: same data pre-cast to fp8 on the host (V projection operand)
    x8_d = nc.declare_dram_parameter("x8", [128, CT * N], D8, isOutput=False)
    # xh: fp32 residual slice (this core's query half): [t, p, i]
    xh_d = nc.declare_dram_parameter("xh", [CT, 128, L], DT, isOutput=False)
    # weights, transposed block layout: [p, t*C + o] = w[o, t*128+p]
    wq_d = nc.declare_dram_parameter("wqT", [128, CT * C], DB, isOutput=False)
    wk_d = nc.declare_dram_parameter("wkT", [128, CT * C], DB, isOutput=False)
    wv_d = nc.declare_dram_parameter("wvT", [128, CT * C], DB, isOutput=False)
    wp_d = nc.declare_dram_parameter("wpT", [128, CT * C], DB, isOutput=False)
    # packed small params: per channel-tile [bq bk bv bp gnw gnb | G(8)] = 14
    bias6_d = nc.declare_dram_parameter("bias6", [CT, 128, 14], DT, isOutput=False)
    gt_d = nc.declare_dram_parameter("GT", [GROUPS, CT * 128], DT, isOutput=False)
    y_d = nc.declare_dram_parameter("y", [CT, 128, L], DT, isOutput=True)

    with tile.TileContext(nc) as tc:
        with (
            tc.tile_pool(name="io", bufs=1) as io,
            tc.tile_pool(name="wp_", bufs=1) as wpool,
            tc.tile_pool(name="kvq", bufs=1) as kvq,
            tc.tile_pool(name="ptp", bufs=3) as ptp,
            tc.tile_pool(name="mis", bufs=2) as mis,
            tc.tile_pool(name="ps_big", bufs=2, space="PSUM") as ps_big,
            tc.tile_pool(name="ps_pv", bufs=1, space="PSUM") as ps_pv,
            tc.tile_pool(name="ps_dn", bufs=1, space="PSUM") as ps_dn,
            tc.tile_pool(name="ps_m", bufs=1, space="PSUM") as ps_m,
        ):
            def body(_it=None):
                # ---------- tiles ----------
                xb_t = io.tile([128, CT * N], DB, tag="xb", name="xb")
                x8_t = io.tile([128, CT * N], D8, tag="x8t", name="x8t")
                xh_t = [io.tile([128, L], DT, tag=f"xh{t}", name=f"xh{t}") for t in range(CT)]
                w_in = {}
                for nm in ("q", "k", "v", "p"):
                    w_in[nm] = io.tile([128, CT * C], DB, tag=f"w{nm}", name=f"w{nm}")
                b6_t = [io.tile([128, 14], DT, tag=f"b6{t}", name=f"b6{t}") for t in range(CT)]
                gt_t = io.tile([GROUPS, CT * 128], DT, tag="gt", name="gt")

                w_s = {nm: kvq.tile([128, CT * C], DB, tag=f"ws{nm}", name=f"ws{nm}")
                       for nm in ("q", "k")}
                w8v = kvq.tile([128, CT * C], D8, tag="w8v", name="w8v")
                w8p = kvq.tile([128, CT * C], D8, tag="w8p", name="w8p")
                k8_t = kvq.tile([128, CT * N], D8, tag="k8", name="k8")
                q8_t = kvq.tile([128, CT * L], D8, tag="q8", name="q8")
                # single vt8 in PSUM-natural layout [p, jt*256 + ct*128 + c']
                vt8_t = kvq.tile([128, NJT * 256], D8, tag="vt8", name="vt8")
                vt8_r = vt8_t[:].rearrange("p (a c) -> p a c", c=128)
                # all-ones DR stationary: the denominator matmul reduces over
                # j AND broadcasts the sum to all 128 output partitions
                ones8 = wpool.tile([128, 256], D8, tag="ones8", name="ones8")
                nc.vector.memset(ones8[:], 1.0)
                shift_t = wpool.tile([128, 1], DT, tag="shift", name="shift")
                nc.vector.memset(shift_t[:], SHIFT)
                eps_t = wpool.tile([GROUPS, 1], DT, tag="eps_t", name="eps_t")
                nc.vector.memset(eps_t[:], EPS)

                # ---------- loads ----------
                # xb split across both queues (first priority), then weights,
                # then the fp32 residual halves (needed only at finales).
                nc.scalar.dma_start(b6_t[0][:], bias6_d[0])
                nc.scalar.dma_start(b6_t[1][:], bias6_d[1])
                nc.scalar.dma_start(gt_t[:], gt_d[:])
                for a in range(8):
                    q = nc.sync if a % 2 == 0 else nc.scalar
                    q.dma_start(xb_t[:, ts(a, 1024)], xb_d[:, ts(a, 1024)])
                for a in range(4):
                    q = nc.sync if a % 2 == 0 else nc.scalar
                    q.dma_start(x8_t[:, ts(a, 2048)], x8_d[:, ts(a, 2048)])
                for i, (nm, wd) in enumerate((("q", wq_d), ("k", wk_d),
                                              ("v", wv_d), ("p", wp_d))):
                    q = nc.sync if i % 2 == 0 else nc.scalar
                    q.dma_start(w_in[nm][:], wd[:])
                for a in range(4):
                    nc.sync.dma_start(xh_t[0][:, ts(a, 512)], xh_d[0, :, ts(a, 512)])
                    nc.scalar.dma_start(xh_t[1][:, ts(a, 512)], xh_d[1, :, ts(a, 512)])

                b_in = {nm: [b6_t[t][:, i:i + 1] for t in range(CT)]
                        for i, nm in enumerate(("q", "k", "v", "p", "gw", "gb"))}
                g_t = [b6_t[t][:, 6:14] for t in range(CT)]

                # HAM warmers: keep the PE activity monitor at full clock
                # through the DMA/stats window.
                for a in range(4):
                    ps_w = ps_m.tile([128, 512], DT, tag="ps_m", name="ps_w")
                    nc.tensor.matmul(
                        ps_w[:], xb_t[:, a * 2048: a * 2048 + 128],
                        xb_t[:, a * 2048: a * 2048 + 512],
                        start=True, stop=True,
                    )

                # ---------- GroupNorm stats (bn_stats, every other column) --
                parts = [wpool.tile([128, 2], DT, tag=f"parts{t}", name=f"parts{t}") for t in range(CT)]
                bns_t = [wpool.tile([128, 4 * 6], DT, tag=f"bns{t}", name=f"bns{t}") for t in range(CT)]
                for a in range(8):
                    t, la = divmod(a, 4)
                    nc.vector.bn_stats(
                        bns_t[t][:, la * 6:(la + 1) * 6],
                        xb_t[:, t * N + la * 1024: t * N + (la + 1) * 1024: 2],
                    )
                for t in range(CT):
                    mv = wpool.tile([128, 2], DT, tag="mv", name=f"mv{t}")
                    nc.vector.bn_aggr(mv[:], bns_t[t][:].rearrange("p (a s) -> p a s", s=6))
                    # parts = [mean_c, ex2_c = var_c + mean_c^2]
                    nc.vector.tensor_mul(parts[t][:, 1:2], mv[:, 0:1], mv[:, 0:1])
                    nc.vector.tensor_add(parts[t][:, 1:2], parts[t][:, 1:2], mv[:, 1:2])
                    nc.vector.tensor_copy(parts[t][:, 0:1], mv[:, 0:1])

                # group stats via indicator matmul (fp32): (8,2) = 32*[mean_g, ex2_g]
                ps_g = ps_m.tile([128, 512], DT, tag="ps_m", name="ps_g")
                for t in range(CT):
                    nc.tensor.matmul(
                        ps_g[:GROUPS, 0:2], g_t[t], parts[t][:],
                        start=(t == 0), stop=(t == CT - 1),
                    )
                st_mv = wpool.tile([GROUPS, 2], DT, tag="st_mv", name="st_mv")
                nc.vector.tensor_scalar_mul(st_mv[:], ps_g[:GROUPS, 0:2], 1.0 / 32)
                st_var = wpool.tile([GROUPS, 1], DT, tag="st_var", name="st_var")
                nc.vector.tensor_mul(st_var[:], st_mv[:, 0:1], st_mv[:, 0:1])
                nc.vector.tensor_sub(st_var[:], st_mv[:, 1:2], st_var[:])
                st2 = wpool.tile([GROUPS, 2], DT, tag="st2", name="st2")
                nc.vector.tensor_copy(st2[:, 0:1], st_mv[:, 0:1])
                st_sd = wpool.tile([GROUPS, 1], DT, tag="st_sd", name="st_sd")
                nc.scalar.activation(st_sd[:], st_var[:], AF.Sqrt, bias=eps_t[:])
                nc.vector.reciprocal(st2[:, 1:2], st_sd[:])

                # broadcast to channels (fp32 matmul): psum (128,2) = GT^T @ st2
                scale_c = [wpool.tile([128, 1], DT, tag=f"scale_c{t}", name=f"scale_c{t}") for t in range(CT)]
                bias_c = [wpool.tile([128, 1], DT, tag=f"bias_c{t}", name=f"bias_c{t}") for t in range(CT)]
                bias_cb = [wpool.tile([128, 1], DB, tag=f"bias_cb{t}", name=f"bias_cb{t}") for t in range(CT)]
                for t in range(CT):
                    ps_bc = ps_m.tile([128, 512], DT, tag="ps_m", name="ps_bc")
                    nc.tensor.matmul(ps_bc[:, 0:2], gt_t[:, ts(t, 128)], st2[:], start=True, stop=True)
                    nc.vector.tensor_mul(scale_c[t][:], b_in["gw"][t], ps_bc[:, 1:2])
                    nc.vector.tensor_mul(bias_c[t][:], ps_bc[:, 0:1], scale_c[t][:])
                    nc.vector.tensor_sub(bias_c[t][:], b_in["gb"][t], bias_c[t][:])
                    nc.vector.tensor_copy(bias_cb[t][:], bias_c[t][:])

                # ---------- fold GN scale into QKV weights ----------
                for nm in ("q", "k"):
                    for t in range(CT):
                        nc.vector.tensor_scalar_mul(
                            w_s[nm][:, ts(t, C)], w_in[nm][:, ts(t, C)], scale_c[t][:]
                        )
                for t in range(CT):
                    nc.vector.tensor_scalar_mul(
                        w8v[:, ts(t, C)], w_in["v"][:, ts(t, C)], scale_c[t][:]
                    )
                nc.vector.tensor_copy(w8p[:], w_in["p"][:])

                # ---------- bias folds (bf16 matmuls, fp32 psum) ----------
                b_f = {}
                for nm in ("q", "k", "v"):
                    b_f[nm] = []
                    for ot in range(CT):
                        ps_f = ps_m.tile([128, 512], DT, tag="ps_m", name="ps_f")
                        for ct in range(CT):
                            nc.tensor.matmul(
                                ps_f[:, 0:1],
                                w_in[nm][:, ct * C + ot * 128: ct * C + ot * 128 + 128],
                                bias_cb[ct][:],
                                start=(ct == 0), stop=(ct == CT - 1),
                            )
                        bf = wpool.tile([128, 1], DT, tag=f"bf{nm}{ot}", name=f"bf{nm}{ot}")
                        nc.vector.tensor_add(bf[:], b_in[nm][ot], ps_f[:, 0:1])
                        b_f[nm].append(bf)
                bfv_b = [wpool.tile([128, 1], DB, tag=f"bfvb{t}", name=f"bfvb{t}") for t in range(CT)]
                for t in range(CT):
                    nc.vector.tensor_copy(bfv_b[t][:], b_f["v"][t][:])
                b_f["p"] = []
                for ot in range(CT):
                    ps_f2 = ps_m.tile([128, 512], DT, tag="ps_m", name="ps_f2")
                    for ct in range(CT):
                        nc.tensor.matmul(
                            ps_f2[:, 0:1],
                            w_in["p"][:, ct * C + ot * 128: ct * C + ot * 128 + 128],
                            bfv_b[ct][:],
                            start=(ct == 0), stop=(ct == CT - 1),
                        )
                    bf = wpool.tile([128, 1], DT, tag=f"bfp{ot}", name=f"bfp{ot}")
                    nc.vector.tensor_add(bf[:], b_in["p"][ot], ps_f2[:, 0:1])
                    b_f["p"].append(bf)

                # ---------- K/Q/V, all pre-attention ----------
                # [128,512]-granular units through a 6-bank psum rotation
                # (ps_big x2 + pspv0 + pspv1 + ps_dn + ps_m are all free
                # before the first attention chunk). Early-needed evacs go
                # to ACT (they gate the first exp anyway, and ACT's queue is
                # in-order); late ones trail on DVE under the exp stream.
                x8r = x8_t[:].rearrange("p (t n) -> p t n", t=CT)
                w8vr = w8v[:].rearrange("p (t o) -> p t o", t=CT)
                qkv_slot = [0]

                def qkv_ps():
                    i = qkv_slot[0] % 6
                    qkv_slot[0] += 1
                    if i < 2:
                        return ps_big.tile([128, 1024], DT, tag="ps_big",
                                           name="ps_qkv")
                    if i < 4:
                        return ps_pv.tile([128, 512], DT, tag=f"pspv{i - 2}",
                                          name="ps_qkv")
                    if i == 4:
                        return ps_dn.tile([128, 512], DT, tag="ps_dn",
                                          name="ps_qkv")
                    return ps_m.tile([128, 512], DT, tag="ps_m", name="ps_qkv")

                def emit_kq_half(nm, dst8, ot, g, s, on_act):
                    base = ot * (N if nm == "k" else L) + g * 1024 + s * 512
                    ps = qkv_ps()
                    for ct in range(CT):
                        nc.tensor.matmul(
                            ps[:, 0:512],
                            w_s[nm][:, ct * C + ot * 128: ct * C + ot * 128 + 128],
                            xb_t[:, ct * N + g * 1024 + s * 512:
                                 ct * N + g * 1024 + (s + 1) * 512],
                            start=(ct == 0), stop=(ct == CT - 1),
                        )
                    if on_act:
                        nc.scalar.activation(dst8[:, base: base + 512], ps[:, 0:512],
                                             AF.Identity, bias=b_f[nm][ot][:])
                    else:
                        nc.vector.tensor_scalar_add(dst8[:, base: base + 512],
                                                    ps[:, 0:512], b_f[nm][ot][:])

                def emit_v_jt(jt, on_act):
                    ps = qkv_ps()
                    nc.tensor.matmul(
                        ps[:, 0:256], x8r[:, :, ts(jt, 128)], w8vr[:, :, :],
                        start=True, stop=True, perf_mode=DRM,
                    )
                    dst = vt8_t[:, jt * 256:(jt + 1) * 256]
                    if on_act:
                        nc.scalar.copy(dst, ps[:, 0:256])
                    else:
                        nc.vector.tensor_copy(dst, ps[:, 0:256])

                for ot in range(CT):            # K g0, Q g0 -> ACT
                    for s in range(2):
                        emit_kq_half("k", k8_t, ot, 0, s, True)
                for ot in range(CT):
                    for s in range(2):
                        emit_kq_half("q", q8_t, ot, 0, s, True)
                for jt in range(8):             # V j-tiles 0-7 -> ACT
                    emit_v_jt(jt, True)
                for g in range(1, 4):           # K g1-3 + V 8-31 -> DVE
                    for ot in range(CT):
                        for s in range(2):
                            emit_kq_half("k", k8_t, ot, g, s, False)
                    for jt in range(8 * g, 8 * (g + 1)):
                        emit_v_jt(jt, False)
                for ot in range(CT):            # Q g1 -> DVE
                    for s in range(2):
                        emit_kq_half("q", q8_t, ot, 1, s, False)

                k8r = k8_t[:].rearrange("p (t n) -> p t n", t=CT)
                q8r = q8_t[:].rearrange("p (t n) -> p t n", t=CT)

                if debug:
                    dmp = io.tile([128, 1024], DT, tag="dmp", name="dmp")
                    dsc = io.tile([128, 4], DT, tag="dsc", name="dsc")
                    for t in range(CT):
                        nc.vector.tensor_copy(dsc[:, t:t + 1], scale_c[t][:])
                        nc.vector.tensor_copy(dsc[:, 2 + t:3 + t], bias_c[t][:])
                    nc.sync.dma_start(dbg_d["d_scale"][:], dsc[:])

                # ---------- attention over i-chunks ----------
                for ic in range(NCH):
                    ps_pv_t = [ps_pv.tile([128, IC], DT, tag=f"pspv{ct}", name=f"pspv{ct}")
                               for ct in range(CT)]
                    ps_den = ps_dn.tile([128, IC], DT, tag="ps_dn", name="ps_den")

                    def emit_s(jp):
                        ps_sc = ps_big.tile([128, 1024], DT, tag="ps_big", name="ps_sc")
                        for q in range(2):
                            nc.tensor.matmul(
                                ps_sc[:, ts(q, 512)],
                                k8r[:, :, ts(2 * jp + q, 128)],
                                q8r[:, :, ic * IC: (ic + 1) * IC],
                                start=True, stop=True, perf_mode=DRM,
                            )
                        pt = ptp.tile([128, 1024], D8, tag="pt", name="pt")
                        nc.scalar.activation(pt[:], ps_sc[:], AF.Exp, scale=SCALE,
                                             bias=shift_t[:])
                        return pt

                    def emit_pv(jp, pt):
                        ptr = pt[:].rearrange("p (q i) -> p q i", q=2)
                        for ct in range(CT):
                            nc.tensor.matmul(
                                ps_pv_t[ct][:],
                                vt8_r[:, 4 * jp + ct: 4 * jp + ct + 3: 2, :],
                                ptr[:, :, :],
                                start=(jp == 0), stop=(jp == NJP - 1),
                                perf_mode=DRM,
                            )
                        nc.tensor.matmul(
                            ps_den[:],
                            ones8[:].rearrange("p (q m) -> p q m", q=2),
                            ptr[:, :, :],
                            start=(jp == 0), stop=(jp == NJP - 1),
                            perf_mode=DRM,
                        )

                    pt_prev = emit_s(0)
                    for jp in range(1, NJP):
                        pt_cur = emit_s(jp)
                        emit_pv(jp - 1, pt_prev)
                        pt_prev = pt_cur
                    emit_pv(NJP - 1, pt_prev)

                    # denominator is already partition-broadcast; reciprocal it
                    rb_sb = mis.tile([128, IC], DT, tag="rb_sb", name="rb_sb")
                    nc.vector.reciprocal(rb_sb[:], ps_den[:])

                    ao8 = mis.tile([128, 2 * IC], D8, tag="ao8", name="ao8")
                    for ct in range(CT):
                        nc.vector.tensor_mul(ao8[:, ts(ct, IC)], ps_pv_t[ct][:], rb_sb[:])

                    if debug and ic == 0:
                        nc.vector.tensor_copy(dmp[:, 0:IC], ps_den[:])
                        nc.sync.dma_start(dbg_d["d_den"][:], dmp[:, 0:IC])
                        nc.vector.tensor_copy(dmp[:], ao8[:])
                        nc.sync.dma_start(dbg_d["d_ao"][:], dmp[:])
                        nc.vector.tensor_copy(dmp[:], pt_prev[:])
                        nc.sync.dma_start(dbg_d["d_pt"][:], dmp[:])

                    ao8r = ao8[:].rearrange("p (t i) -> p t i", t=CT)
                    w8pr = w8p[:].rearrange("p (t o) -> p t o", t=CT)
                    for ot in range(CT):
                        ps_y = ps_m.tile([128, 512], DT, tag="ps_m", name="ps_y")
                        nc.tensor.matmul(
                            ps_y[:], w8pr[:, :, ts(ot, 128)],
                            ao8r[:, :, :],
                            start=True, stop=True, perf_mode=DRM,
                        )
                        y_sb = mis.tile([128, IC], DT, tag="y_sb", name="y_sb")
                        nc.vector.scalar_tensor_tensor(
                            y_sb[:], ps_y[:], b_f["p"][ot][:],
                            xh_t[ot][:, ts(ic, IC)],
                            op0=OP.add, op1=OP.add,
                        )
                        nc.sync.dma_start(y_d[ot, :, ts(ic, IC)], y_sb[:])

                if debug:
                    for a in range(8):
                        nc.vector.tensor_copy(dmp[:], k8_t[:, ts(a, 1024)])
                        nc.sync.dma_start(dbg_d["d_k8"][:, ts(a, 1024)], dmp[:])
                    for a in range(4):
                        nc.vector.tensor_copy(dmp[:], q8_t[:, ts(a, 1024)])
                        nc.sync.dma_start(dbg_d["d_q8"][:, ts(a, 1024)], dmp[:])
                    nc.vector.tensor_copy(dmp[:], vt8_t[:, 0:1024])
                    nc.sync.dma_start(dbg_d["d_vt8"][:], dmp[:])

            if repeat == 1:
                body()
            else:
                hints = (mybir.EngineType.PE, mybir.EngineType.Activation,
                         mybir.EngineType.DVE, mybir.EngineType.SP)
                with tc.For_i(0, repeat, 1, hint_engines=hints) as it:
                    body(it)

    if split:
        split_waits(nc)
    return nc


# ---------------- host-side sharding helpers ----------------

def make_in_maps(inputs):
    fb = mybir.dt.np(DB)

    x = np.asarray(inputs["x"], dtype=np.float32)
    n = x.shape[0]

    def wt(name):
        w = np.asarray(inputs[name], dtype=np.float32)
        # [p, t*C + o] = w[o, t*128+p]
        return np.ascontiguousarray(
            w.T.reshape(CT, 128, C).transpose(1, 0, 2).reshape(128, CT * C)
        ).astype(fb)

    wqT, wkT, wvT, wpT = wt("wq"), wt("wk"), wt("wv"), wt("wp")

    bias6 = np.zeros((CT, 128, 14), dtype=np.float32)
    for i, nm in enumerate(("bq", "bk", "bv", "bp", "gn_w", "gn_b")):
        bias6[:, :, i] = np.asarray(inputs[nm], np.float32).reshape(CT, 128)
    for t in range(CT):
        for p in range(128):
            bias6[t, p, 6 + (t * 128 + p) // 32] = 1.0  # G indicator
    GT = np.zeros((GROUPS, CT * 128), dtype=np.float32)
    for c in range(C):
        GT[c // 32, c] = 1.0

    in_maps = []
    xb_cache = {}
    for core in range(2 * n):
        b, h = divmod(core, 2)
        xb = x[b].reshape(CT, 128, N)
        key = (b, h)
        if key not in xb_cache:
            # pre-rolled so the program's query columns [0, L) are this
            # half's queries; keys are permutation-invariant
            xr = np.roll(xb, -h * L, axis=2) if h else xb
            flat = np.ascontiguousarray(
                xr.transpose(1, 0, 2).reshape(128, CT * N))
            xb_cache[key] = (flat.astype(fb), flat.astype(mybir.dt.np(D8)))
        xh = np.ascontiguousarray(xb[:, :, h * L:(h + 1) * L])
        in_maps.append({
            "xb": xb_cache[key][0],
            "x8": xb_cache[key][1],
            "xh": xh,
            "wqT": wqT, "wkT": wkT, "wvT": wvT, "wpT": wpT,
            "bias6": bias6, "GT": GT,
        })
    return in_maps


def assemble(results, n=4):
    out = np.zeros((n, C, 64, 64), dtype=np.float32)
    flat = out.reshape(n, C, N)
    for core, res in enumerate(results):
        b, h = divmod(core, 2)
        flat[b, :, h * L:(h + 1) * L] = res["y"].reshape(C, L)
    return out


_CACHE = {}


def kernel(**inputs) -> np.ndarray:
    n = np.asarray(inputs["x"]).shape[0]
    n_cores = 2 * n
    if "nc" not in _CACHE:
        _CACHE["nc"] = build(split=True, repeat=1)
    nc = _CACHE["nc"]
    in_maps = make_in_maps(inputs)
    last_err = None
    for _attempt in range(2):  # one retry on transient axon/RPC failures
        try:
            res = run_bass_kernel_spmd(nc, in_maps, list(range(n_cores)))
            return assemble(res.results, n=n)
        except Exception as e:  # noqa: BLE001
            last_err = e
    raise last_err


# revision 3
# speedup vs baseline: 1.1776x; 1.1776x over previous
"""AttentionBlock (GroupNorm + single-head self-attention + residual) as a
Bass/Tile kernel for one Trainium2 chip (8 NeuronCores), SPMD data-parallel.

fp8/bf16 redesign of the f32r baseline:
- QKV projections run in bf16 (x and weights bf16; fp8 here would push the
  worst-case rel err near the 2e-2 gate on wide-score datasets). Their
  outputs are cast to fp8e4 at PSUM evacuation.
- Attention matmuls (scores, PV, out-proj) run in float8e4 (e4m3) with
  MatmulPerfMode.DoubleRow: the 256-deep contraction is packed as two
  128-row matrices per call at 0.5 PE cycles/row.
- Softmax over the partition axis (S^T layout): exp on ACT in [128,1024]
  batches straight out of paired PSUM banks; exp arg is SCALE*s - 3.5 so
  fp8 pt can't overflow (ACT's fp8 cast yields inf, not saturation, above
  240; shift-invariance keeps softmax exact). The denominator is an
  all-ones DoubleRow matmul accumulated on PE alongside PV — it reduces
  over j AND broadcasts the sum to all 128 partitions (replaces the
  baseline's DVE accumulation tree + ones_sq broadcast matmul).
- GroupNorm stats come from every other column of the bf16 x copy
  (sampling error ~0.3% of sigma, far below tolerance) so the stats chain
  clears ~4us after the x DMA. rsqrt = ACT Sqrt + DVE reciprocal;
  identity/copy live in every ACT table so the only table switch is
  sqrt -> exp, once.
- V's bias is never applied on-chip: sum_j attn = 1, so it folds into the
  output-projection bias (b_fp = bp + wp.T @ (bv + wv.T @ gn_bias)).
- K (g-major) and Q stream group-wise; attention chunk 0 starts once the
  first K/Q groups land, and the remaining K/Q/V work is interleaved into
  the chunk-0/1 pair slots (PE has ~2x slack under the ACT exp stream).

Sharding: 4 images x 2 query-halves -> 8 cores. x is pre-rolled per half
on the host (keys are permutation-invariant); residual/output use the
original column range h*L..(h+1)*L.
"""

import numpy as np

import bass_rust
import concourse.bass as bass
import concourse.mybir as mybir
import concourse.tile as tile
from concourse.bass import ts
from concourse.bass_utils import run_bass_kernel_spmd

# ---------------------------------------------------------------------------
# walrus single-sync-wait workaround (same as baseline)

_counter = [0]


def _mk_nop(engine, wait):
    _counter[0] += 1
    nop = mybir.InstNoOp(name=f"WSPLIT-{_counter[0]}", ins=[], outs=[])
    nop.engine = engine
    nop.sync_info = bass_rust.SyncInfo(on_wait=[wait], on_update=[])
    return nop


def split_waits(nc, verbose=False):
    f = nc.m.functions[0]
    new_blocks = []
    n_split = 0
    for blk in f.blocks:
        insts = blk.instructions
        out = []
        for inst in insts:
            si = inst.sync_info
            if si is not None and si.on_wait and len(si.on_wait) > 1:
                waits = list(si.on_wait)
                for w in waits[1:]:
                    out.append(_mk_nop(inst.engine, w))
                si.on_wait = waits[:1]
                n_split += 1
            out.append(inst)
        new_blocks.append(bass_rust.BasicBlock(name=blk.name, instructions=out))
    f.blocks = new_blocks
    if verbose:
        print(f"split_waits: split {n_split} instructions")
    return n_split


# ---------------------------------------------------------------------------

DT = mybir.dt.float32
DB = mybir.dt.bfloat16
D8 = mybir.dt.float8e4
AF = mybir.ActivationFunctionType
OP = mybir.AluOpType
DRM = mybir.MatmulPerfMode.DoubleRow

C = 256
N = 4096
L = 2048
IC = 512          # i-chunk size
NCH = L // IC     # 4 chunks
NJT = N // 128    # 32 j-tiles
NJP = NJT // 2    # 16 j-tile pairs
CT = C // 128     # 2 channel tiles
GROUPS = 8
EPS = 1e-5
SCALE = C ** -0.5
SHIFT = -3.5


def build(split=True, repeat=1, prec=None, debug=False):
    nc = bass.Bass()
    dbg_d = {}
    if debug:
        dbg_d["d_scale"] = nc.declare_dram_parameter("d_scale", [128, 2 * CT], DT, isOutput=True)
        dbg_d["d_k8"] = nc.declare_dram_parameter("d_k8", [128, CT * N], DT, isOutput=True)
        dbg_d["d_q8"] = nc.declare_dram_parameter("d_q8", [128, CT * L], DT, isOutput=True)
        dbg_d["d_vt8"] = nc.declare_dram_parameter("d_vt8", [128, 2 * 512], DT, isOutput=True)
        dbg_d["d_pt"] = nc.declare_dram_parameter("d_pt", [128, 1024], DT, isOutput=True)
        dbg_d["d_den"] = nc.declare_dram_parameter("d_den", [128, IC], DT, isOutput=True)
        dbg_d["d_ao"] = nc.declare_dram_parameter("d_ao", [128, 2 * IC], DT, isOutput=True)

    # xb: full image, bf16, ct-major free dim: [p, t*N + j] = x[t*128+p, j]
    xb_d = nc.declare_dram_parameter("xb", [128, CT * N], DB, isOutput=False)
    # x8: same data pre-cast to fp8 on the host (V projection operand)
    x8_d = nc.declare_dram_parameter("x8", [128, CT * N], D8, isOutput=False)
    # xh: fp32 residual slice (this core's query half): [t, p, i]
    xh_d = nc.declare_dram_parameter("xh", [CT, 128, L], DT, isOutput=False)
    # weights, transposed block layout: [p, t*C + o] = w[o, t*128+p]
    wq_d = nc.declare_dram_parameter("wqT", [128, CT * C], DB, isOutput=False)
    wk_d = nc.declare_dram_parameter("wkT", [128, CT * C], DB, isOutput=False)
    wv_d = nc.declare_dram_parameter("wvT", [128, CT * C], DB, isOutput=False)
    wp_d = nc.declare_dram_parameter("wpT", [128, CT * C], DB, isOutput=False)
    # packed small params: per channel-tile [bq bk bv bp gnw gnb | G(8)] = 14
    bias6_d = nc.declare_dram_parameter("bias6", [CT, 128, 14], DT, isOutput=False)
    gt_d = nc.declare_dram_parameter("GT", [GROUPS, CT * 128], DT, isOutput=False)
    y_d = nc.declare_dram_parameter("y", [CT, 128, L], DT, isOutput=True)

    with tile.TileContext(nc) as tc:
        with (
            tc.tile_pool(name="io", bufs=1) as io,
            tc.tile_pool(name="wp_", bufs=1) as wpool,
            tc.tile_pool(name="kvq", bufs=1) as kvq,
            tc.tile_pool(name="ptp", bufs=4) as ptp,
            tc.tile_pool(name="mis", bufs=3) as mis,
            tc.tile_pool(name="ps_big", bufs=2, space="PSUM") as ps_big,
            tc.tile_pool(name="ps_pv", bufs=1, space="PSUM") as ps_pv,
            tc.tile_pool(name="ps_dn", bufs=1, space="PSUM") as ps_dn,
            tc.tile_pool(name="ps_m", bufs=1, space="PSUM") as ps_m,
        ):
            def body(_it=None):
                # ---------- tiles ----------
                xb_t = io.tile([128, CT * N], DB, tag="xb", name="xb")
                x8_t = io.tile([128, CT * N], D8, tag="x8t", name="x8t")
                xh_t = [io.tile([128, L], DT, tag=f"xh{t}", name=f"xh{t}") for t in range(CT)]
                w_in = {}
                for nm in ("q", "k", "v", "p"):
                    w_in[nm] = io.tile([128, CT * C], DB, tag=f"w{nm}", name=f"w{nm}")
                b6_t = [io.tile([128, 14], DT, tag=f"b6{t}", name=f"b6{t}") for t in range(CT)]
                gt_t = io.tile([GROUPS, CT * 128], DT, tag="gt", name="gt")

                w_s = {nm: kvq.tile([128, CT * C], DB, tag=f"ws{nm}", name=f"ws{nm}")
                       for nm in ("q", "k")}
                w8v = kvq.tile([128, CT * C], D8, tag="w8v", name="w8v")
                w8p = kvq.tile([128, CT * C], D8, tag="w8p", name="w8p")
                k8_t = kvq.tile([128, CT * N], D8, tag="k8", name="k8")
                q8_t = kvq.tile([128, CT * L], D8, tag="q8", name="q8")
                # single vt8 in PSUM-natural layout [p, jt*256 + ct*128 + c']
                vt8_t = kvq.tile([128, NJT * 256], D8, tag="vt8", name="vt8")
                vt8_r = vt8_t[:].rearrange("p (a c) -> p a c", c=128)
                # all-ones DR stationary: the denominator matmul reduces over
                # j AND broadcasts the sum to all 128 output partitions
                ones8 = wpool.tile([128, 256], D8, tag="ones8", name="ones8")
                nc.vector.memset(ones8[:], 1.0)
                shift_t = wpool.tile([128, 1], DT, tag="shift", name="shift")
                nc.vector.memset(shift_t[:], SHIFT)
                eps_t = wpool.tile([GROUPS, 1], DT, tag="eps_t", name="eps_t")
                nc.vector.memset(eps_t[:], EPS)

                # ---------- loads ----------
                # xb split across both queues (first priority), then weights,
                # then the fp32 residual halves (needed only at finales).
                nc.scalar.dma_start(b6_t[0][:], bias6_d[0])
                nc.scalar.dma_start(b6_t[1][:], bias6_d[1])
                nc.scalar.dma_start(gt_t[:], gt_d[:])
                for a in range(8):
                    q = nc.sync if a % 2 == 0 else nc.scalar
                    q.dma_start(xb_t[:, ts(a, 1024)], xb_d[:, ts(a, 1024)])
                for a in range(4):
                    q = nc.sync if a % 2 == 0 else nc.scalar
                    q.dma_start(x8_t[:, ts(a, 2048)], x8_d[:, ts(a, 2048)])
                for i, (nm, wd) in enumerate((("q", wq_d), ("k", wk_d),
                                              ("v", wv_d), ("p", wp_d))):
                    q = nc.sync if i % 2 == 0 else nc.scalar
                    q.dma_start(w_in[nm][:], wd[:])
                for a in range(4):
                    nc.sync.dma_start(xh_t[0][:, ts(a, 512)], xh_d[0, :, ts(a, 512)])
                    nc.scalar.dma_start(xh_t[1][:, ts(a, 512)], xh_d[1, :, ts(a, 512)])

                b_in = {nm: [b6_t[t][:, i:i + 1] for t in range(CT)]
                        for i, nm in enumerate(("q", "k", "v", "p", "gw", "gb"))}
                g_t = [b6_t[t][:, 6:14] for t in range(CT)]

                # HAM warmers: keep the PE activity monitor at full clock
                # through the DMA/stats window.
                for a in range(4):
                    ps_w = ps_m.tile([128, 512], DT, tag="ps_m", name="ps_w")
                    nc.tensor.matmul(
                        ps_w[:], xb_t[:, a * 2048: a * 2048 + 128],
                        xb_t[:, a * 2048: a * 2048 + 512],
                        start=True, stop=True,
                    )

                # ---------- GroupNorm stats (bn_stats, every other column) --
                parts = [wpool.tile([128, 2], DT, tag=f"parts{t}", name=f"parts{t}") for t in range(CT)]
                bns_t = [wpool.tile([128, 4 * 6], DT, tag=f"bns{t}", name=f"bns{t}") for t in range(CT)]
                for a in range(8):
                    t, la = divmod(a, 4)
                    nc.vector.bn_stats(
                        bns_t[t][:, la * 6:(la + 1) * 6],
                        xb_t[:, t * N + la * 1024: t * N + (la + 1) * 1024: 2],
                    )
                for t in range(CT):
                    mv = wpool.tile([128, 2], DT, tag="mv", name=f"mv{t}")
                    nc.vector.bn_aggr(mv[:], bns_t[t][:].rearrange("p (a s) -> p a s", s=6))
                    # parts = [mean_c, ex2_c = var_c + mean_c^2]
                    nc.vector.tensor_mul(parts[t][:, 1:2], mv[:, 0:1], mv[:, 0:1])
                    nc.vector.tensor_add(parts[t][:, 1:2], parts[t][:, 1:2], mv[:, 1:2])
                    nc.vector.tensor_copy(parts[t][:, 0:1], mv[:, 0:1])

                # group stats via indicator matmul (fp32): (8,2) = 32*[mean_g, ex2_g]
                ps_g = ps_m.tile([128, 512], DT, tag="ps_m", name="ps_g")
                for t in range(CT):
                    nc.tensor.matmul(
                        ps_g[:GROUPS, 0:2], g_t[t], parts[t][:],
                        start=(t == 0), stop=(t == CT - 1),
                    )
                st_mv = wpool.tile([GROUPS, 2], DT, tag="st_mv", name="st_mv")
                nc.vector.tensor_scalar_mul(st_mv[:], ps_g[:GROUPS, 0:2], 1.0 / 32)
                st_var = wpool.tile([GROUPS, 1], DT, tag="st_var", name="st_var")
                nc.vector.tensor_mul(st_var[:], st_mv[:, 0:1], st_mv[:, 0:1])
                nc.vector.tensor_sub(st_var[:], st_mv[:, 1:2], st_var[:])
                st2 = wpool.tile([GROUPS, 2], DT, tag="st2", name="st2")
                nc.vector.tensor_copy(st2[:, 0:1], st_mv[:, 0:1])
                st_sd = wpool.tile([GROUPS, 1], DT, tag="st_sd", name="st_sd")
                nc.scalar.activation(st_sd[:], st_var[:], AF.Sqrt, bias=eps_t[:])
                nc.vector.reciprocal(st2[:, 1:2], st_sd[:])

                # broadcast to channels (fp32 matmul): psum (128,2) = GT^T @ st2
                scale_c = [wpool.tile([128, 1], DT, tag=f"scale_c{t}", name=f"scale_c{t}") for t in range(CT)]
                bias_c = [wpool.tile([128, 1], DT, tag=f"bias_c{t}", name=f"bias_c{t}") for t in range(CT)]
                bias_cb = [wpool.tile([128, 1], DB, tag=f"bias_cb{t}", name=f"bias_cb{t}") for t in range(CT)]
                for t in range(CT):
                    ps_bc = ps_m.tile([128, 512], DT, tag="ps_m", name="ps_bc")
                    nc.tensor.matmul(ps_bc[:, 0:2], gt_t[:, ts(t, 128)], st2[:], start=True, stop=True)
                    nc.vector.tensor_mul(scale_c[t][:], b_in["gw"][t], ps_bc[:, 1:2])
                    nc.vector.tensor_mul(bias_c[t][:], ps_bc[:, 0:1], scale_c[t][:])
                    nc.vector.tensor_sub(bias_c[t][:], b_in["gb"][t], bias_c[t][:])
                    nc.vector.tensor_copy(bias_cb[t][:], bias_c[t][:])

                # ---------- fold GN scale into QKV weights ----------
                for nm in ("q", "k"):
                    for t in range(CT):
                        nc.vector.tensor_scalar_mul(
                            w_s[nm][:, ts(t, C)], w_in[nm][:, ts(t, C)], scale_c[t][:]
                        )
                for t in range(CT):
                    nc.vector.tensor_scalar_mul(
                        w8v[:, ts(t, C)], w_in["v"][:, ts(t, C)], scale_c[t][:]
                    )
                nc.vector.tensor_copy(w8p[:], w_in["p"][:])

                # ---------- bias folds (bf16 matmuls, fp32 psum) ----------
                b_f = {}
                for nm in ("q", "k", "v"):
                    b_f[nm] = []
                    for ot in range(CT):
                        ps_f = ps_m.tile([128, 512], DT, tag="ps_m", name="ps_f")
                        for ct in range(CT):
                            nc.tensor.matmul(
                                ps_f[:, 0:1],
                                w_in[nm][:, ct * C + ot * 128: ct * C + ot * 128 + 128],
                                bias_cb[ct][:],
                                start=(ct == 0), stop=(ct == CT - 1),
                            )
                        bf = wpool.tile([128, 1], DT, tag=f"bf{nm}{ot}", name=f"bf{nm}{ot}")
                        nc.vector.tensor_add(bf[:], b_in[nm][ot], ps_f[:, 0:1])
                        b_f[nm].append(bf)
                bfv_b = [wpool.tile([128, 1], DB, tag=f"bfvb{t}", name=f"bfvb{t}") for t in range(CT)]
                for t in range(CT):
                    nc.vector.tensor_copy(bfv_b[t][:], b_f["v"][t][:])
                b_f["p"] = []
                for ot in range(CT):
                    ps_f2 = ps_m.tile([128, 512], DT, tag="ps_m", name="ps_f2")
                    for ct in range(CT):
                        nc.tensor.matmul(
                            ps_f2[:, 0:1],
                            w_in["p"][:, ct * C + ot * 128: ct * C + ot * 128 + 128],
                            bfv_b[ct][:],
                            start=(ct == 0), stop=(ct == CT - 1),
                        )
                    bf = wpool.tile([128, 1], DT, tag=f"bfp{ot}", name=f"bfp{ot}")
                    nc.vector.tensor_add(bf[:], b_in["p"][ot], ps_f2[:, 0:1])
                    b_f["p"].append(bf)

                # ---------- K/Q/V, all pre-attention ----------
                # [128,512]-granular units through a 6-bank psum rotation
                # (ps_big x2 + pspv0 + pspv1 + ps_dn + ps_m are all free
                # before the first attention chunk). Early-needed evacs go
                # to ACT (they gate the first exp anyway, and ACT's queue is
                # in-order); late ones trail on DVE under the exp stream.
                x8r = x8_t[:].rearrange("p (t n) -> p t n", t=CT)
                w8vr = w8v[:].rearrange("p (t o) -> p t o", t=CT)
                qkv_slot = [0]

                def qkv_ps():
                    i = qkv_slot[0] % 6
                    qkv_slot[0] += 1
                    if i < 2:
                        return ps_big.tile([128, 1024], DT, tag="ps_big",
                                           name="ps_qkv")
                    if i < 4:
                        return ps_pv.tile([128, 512], DT, tag=f"pspv{i - 2}",
                                          name="ps_qkv")
                    if i == 4:
                        return ps_dn.tile([128, 512], DT, tag="ps_dn",
                                          name="ps_qkv")
                    return ps_m.tile([128, 512], DT, tag="ps_m", name="ps_qkv")

                def emit_kq_half(nm, dst8, ot, g, s, on_act):
                    base = ot * (N if nm == "k" else L) + g * 1024 + s * 512
                    ps = qkv_ps()
                    for ct in range(CT):
                        nc.tensor.matmul(
                            ps[:, 0:512],
                            w_s[nm][:, ct * C + ot * 128: ct * C + ot * 128 + 128],
                            xb_t[:, ct * N + g * 1024 + s * 512:
                                 ct * N + g * 1024 + (s + 1) * 512],
                            start=(ct == 0), stop=(ct == CT - 1),
                        )
                    if on_act:
                        nc.scalar.activation(dst8[:, base: base + 512], ps[:, 0:512],
                                             AF.Identity, bias=b_f[nm][ot][:])
                    else:
                        nc.vector.tensor_scalar_add(dst8[:, base: base + 512],
                                                    ps[:, 0:512], b_f[nm][ot][:])

                def emit_v_jt(jt, on_act):
                    ps = qkv_ps()
                    nc.tensor.matmul(
                        ps[:, 0:256], x8r[:, :, ts(jt, 128)], w8vr[:, :, :],
                        start=True, stop=True, perf_mode=DRM,
                    )
                    dst = vt8_t[:, jt * 256:(jt + 1) * 256]
                    if on_act:
                        nc.scalar.copy(dst, ps[:, 0:256])
                    else:
                        nc.vector.tensor_copy(dst, ps[:, 0:256])

                for ot in range(CT):            # K g0, Q g0 -> ACT
                    for s in range(2):
                        emit_kq_half("k", k8_t, ot, 0, s, True)
                for ot in range(CT):
                    for s in range(2):
                        emit_kq_half("q", q8_t, ot, 0, s, True)
                for jt in range(8):             # V j-tiles 0-7 -> ACT
                    emit_v_jt(jt, True)
                for g in range(1, 4):           # K g1-3 + V 8-31 -> DVE
                    for ot in range(CT):
                        for s in range(2):
                            emit_kq_half("k", k8_t, ot, g, s, False)
                    for jt in range(8 * g, 8 * (g + 1)):
                        emit_v_jt(jt, False)
                for ot in range(CT):            # Q g1 -> DVE
                    for s in range(2):
                        emit_kq_half("q", q8_t, ot, 1, s, False)

                k8r = k8_t[:].rearrange("p (t n) -> p t n", t=CT)
                q8r = q8_t[:].rearrange("p (t n) -> p t n", t=CT)

                if debug:
                    dmp = io.tile([128, 1024], DT, tag="dmp", name="dmp")
                    dsc = io.tile([128, 4], DT, tag="dsc", name="dsc")
                    for t in range(CT):
                        nc.vector.tensor_copy(dsc[:, t:t + 1], scale_c[t][:])
                        nc.vector.tensor_copy(dsc[:, 2 + t:3 + t], bias_c[t][:])
                    nc.sync.dma_start(dbg_d["d_scale"][:], dsc[:])

                # ---------- attention over i-chunks ----------
                for ic in range(NCH):
                    ps_pv_t = [ps_pv.tile([128, IC], DT, tag=f"pspv{ct}", name=f"pspv{ct}")
                               for ct in range(CT)]
                    ps_den = ps_dn.tile([128, IC], DT, tag="ps_dn", name="ps_den")

                    def emit_s(jp):
                        ps_sc = ps_big.tile([128, 1024], DT, tag="ps_big", name="ps_sc")
                        for q in range(2):
                            nc.tensor.matmul(
                                ps_sc[:, ts(q, 512)],
                                k8r[:, :, ts(2 * jp + q, 128)],
                                q8r[:, :, ic * IC: (ic + 1) * IC],
                                start=True, stop=True, perf_mode=DRM,
                            )
                        pt = ptp.tile([128, 1024], D8, tag="pt", name="pt")
                        nc.scalar.activation(pt[:], ps_sc[:], AF.Exp, scale=SCALE,
                                             bias=shift_t[:])
                        return pt

                    def emit_pv(jp, pt):
                        ptr = pt[:].rearrange("p (q i) -> p q i", q=2)
                        for ct in range(CT):
                            nc.tensor.matmul(
                                ps_pv_t[ct][:],
                                vt8_r[:, 4 * jp + ct: 4 * jp + ct + 3: 2, :],
                                ptr[:, :, :],
                                start=(jp == 0), stop=(jp == NJP - 1),
                                perf_mode=DRM,
                            )
                        nc.tensor.matmul(
                            ps_den[:],
                            ones8[:].rearrange("p (q m) -> p q m", q=2),
                            ptr[:, :, :],
                            start=(jp == 0), stop=(jp == NJP - 1),
                            perf_mode=DRM,
                        )

                    pt_prev = emit_s(0)
                    for jp in range(1, NJP):
                        pt_cur = emit_s(jp)
                        emit_pv(jp - 1, pt_prev)
                        pt_prev = pt_cur
                    emit_pv(NJP - 1, pt_prev)

                    # denominator is already partition-broadcast; reciprocal it
                    rb_sb = mis.tile([128, IC], DT, tag="rb_sb", name="rb_sb")
                    nc.vector.reciprocal(rb_sb[:], ps_den[:])

                    ao8 = mis.tile([128, 2 * IC], D8, tag="ao8", name="ao8")
                    for ct in range(CT):
                        nc.vector.tensor_mul(ao8[:, ts(ct, IC)], ps_pv_t[ct][:], rb_sb[:])

                    if debug and ic == 0:
                        nc.vector.tensor_copy(dmp[:, 0:IC], ps_den[:])
                        nc.sync.dma_start(dbg_d["d_den"][:], dmp[:, 0:IC])
                        nc.vector.tensor_copy(dmp[:], ao8[:])
                        nc.sync.dma_start(dbg_d["d_ao"][:], dmp[:])
                        nc.vector.tensor_copy(dmp[:], pt_prev[:])
                        nc.sync.dma_start(dbg_d["d_pt"][:], dmp[:])

                    ao8r = ao8[:].rearrange("p (t i) -> p t i", t=CT)
                    w8pr = w8p[:].rearrange("p (t o) -> p t o", t=CT)
                    for ot in range(CT):
                        ps_y = ps_m.tile([128, 512], DT, tag="ps_m", name="ps_y")
                        nc.tensor.matmul(
                            ps_y[:], w8pr[:, :, ts(ot, 128)],
                            ao8r[:, :, :],
                            start=True, stop=True, perf_mode=DRM,
                        )
                        y_sb = mis.tile([128, IC], DT, tag="y_sb", name="y_sb")
                        nc.vector.scalar_tensor_tensor(
                            y_sb[:], ps_y[:], b_f["p"][ot][:],
                            xh_t[ot][:, ts(ic, IC)],
                            op0=OP.add, op1=OP.add,
                        )
                        nc.sync.dma_start(y_d[ot, :, ts(ic, IC)], y_sb[:])

                if debug:
                    for a in range(8):
                        nc.vector.tensor_copy(dmp[:], k8_t[:, ts(a, 1024)])
                        nc.sync.dma_start(dbg_d["d_k8"][:, ts(a, 1024)], dmp[:])
                    for a in range(4):
                        nc.vector.tensor_copy(dmp[:], q8_t[:, ts(a, 1024)])
                        nc.sync.dma_start(dbg_d["d_q8"][:, ts(a, 1024)], dmp[:])
                    nc.vector.tensor_copy(dmp[:], vt8_t[:, 0:1024])
                    nc.sync.dma_start(dbg_d["d_vt8"][:], dmp[:])

            if repeat == 1:
                body()
            else:
                hints = (mybir.EngineType.PE, mybir.EngineType.Activation,
                         mybir.EngineType.DVE, mybir.EngineType.SP)
                with tc.For_i(0, repeat, 1, hint_engines=hints) as it:
                    body(it)

    if split:
        split_waits(nc)
    return nc


# ---------------- host-side sharding helpers ----------------

def make_in_maps(inputs):
    fb = mybir.dt.np(DB)

    x = np.asarray(inputs["x"], dtype=np.float32)
    n = x.shape[0]

    def wt(name):
        w = np.asarray(inputs[name], dtype=np.float32)
        # [p, t*C + o] = w[o, t*128+p]
        return np.ascontiguousarray(
            w.T.reshape(CT, 128, C).transpose(1, 0, 2).reshape(128, CT * C)
        ).astype(fb)

    wqT, wkT, wvT, wpT = wt("wq"), wt("wk"), wt("wv"), wt("wp")

    bias6 = np.zeros((CT, 128, 14), dtype=np.float32)
    for i, nm in enumerate(("bq", "bk", "bv", "bp", "gn_w", "gn_b")):
        bias6[:, :, i] = np.asarray(inputs[nm], np.float32).reshape(CT, 128)
    for t in range(CT):
        for p in range(128):
            bias6[t, p, 6 + (t * 128 + p) // 32] = 1.0  # G indicator
    GT = np.zeros((GROUPS, CT * 128), dtype=np.float32)
    for c in range(C):
        GT[c // 32, c] = 1.0

    in_maps = []
    xb_cache = {}
    for core in range(2 * n):
        b, h = divmod(core, 2)
        xb = x[b].reshape(CT, 128, N)
        key = (b, h)
        if key not in xb_cache:
            # pre-rolled so the program's query columns [0, L) are this
            # half's queries; keys are permutation-invariant
            xr = np.roll(xb, -h * L, axis=2) if h else xb
            flat = np.ascontiguousarray(
                xr.transpose(1, 0, 2).reshape(128, CT * N))
            xb_cache[key] = (flat.astype(fb), flat.astype(mybir.dt.np(D8)))
        xh = np.ascontiguousarray(xb[:, :, h * L:(h + 1) * L])
        in_maps.append({
            "xb": xb_cache[key][0],
            "x8": xb_cache[key][1],
            "xh": xh,
            "wqT": wqT, "wkT": wkT, "wvT": wvT, "wpT": wpT,
            "bias6": bias6, "GT": GT,
        })
    return in_maps


def assemble(results, n=4):
    out = np.zeros((n, C, 64, 64), dtype=np.float32)
    flat = out.reshape(n, C, N)
    for core, res in enumerate(results):
        b, h = divmod(core, 2)
        flat[b, :, h * L:(h + 1) * L] = res["y"].reshape(C, L)
    return out


_CACHE = {}


def kernel(**inputs) -> np.ndarray:
    n = np.asarray(inputs["x"]).shape[0]
    n_cores = 2 * n
    if "nc" not in _CACHE:
        _CACHE["nc"] = build(split=True, repeat=1)
    nc = _CACHE["nc"]
    in_maps = make_in_maps(inputs)
    last_err = None
    for _attempt in range(2):  # one retry on transient axon/RPC failures
        try:
            res = run_bass_kernel_spmd(nc, in_maps, list(range(n_cores)))
            return assemble(res.results, n=n)
        except Exception as e:  # noqa: BLE001
            last_err = e
    raise last_err
